# revision 80
# baseline (speedup 1.0000x reference)
"""MLA forward kernel for Trainium2, 8 NeuronCores.

Sharding: 2 batch groups x 4 head groups. Core c handles batch b=c//4 and
heads 4g..4g+3 where g=c%4. Each core computes the LoRA down-projections for
its batch (replicated within the batch group), its 4 heads' attention, and a
partial output projection (contraction over its heads' value dims). The host
sums the 4 partials per batch and adds the output bias.

All device matmuls run in bf16 (fp32 PSUM accumulation); layout is
feature-major (features on partitions, tokens on free dim) throughout.
RoPE rotate-half is a PE permutation matmul with the rotation signs folded
into the host-precomputed sin table. Causal softmax runs without max
subtraction (scores are bounded by construction); exp row-sums come from the
scalar engine's accum_out.

Schedule notes: the q and kv LoRA projections share x tiles in one fused
pass; LN mean/rstd rows are broadcast across partitions with a PE
ones-matmul into PSUM (no DRAM roundtrip) so the in-place LN apply pipelines
per 512-token tile; big weight loads ride the scalar-DMA queue (x tiles on
the sync queue) and are laid out flat per partition so each load is 128
single-run descriptors; the P4 weight load is prefetched during attention.
The "dep"/"dep_out" passthrough tensors exist so the timing harness can
serially chain many kernel executions inside one jitted dispatch chain.
"""
import sys

sys.path.insert(0, "/opt/trn_rl_repo")

import math
from contextlib import ExitStack

import numpy as np
import ml_dtypes

import concourse.bacc as bacc
import concourse.bass as bass
import concourse.tile as tile
from concourse import mybir
from concourse.bass_utils import run_bass_kernel_spmd
from concourse.masks import make_identity

F32 = mybir.dt.float32
BF16 = mybir.dt.bfloat16
AF = mybir.ActivationFunctionType
ALU = mybir.AluOpType
BF = ml_dtypes.bfloat16

B, T, DIM = 2, 2048, 2048
H, QLR, KVLR = 16, 1024, 512
DN, DR, DV = 128, 64, 128
DQK = DN + DR
EPS = 1e-5
HPG = 4          # heads per group (per core)
NCORES = 8
SCALE = 1.0 / math.sqrt(DQK)
NT = T // 512    # 512-wide token tiles
NQT = T // 128   # 128-row query tiles
MASK_NEG = -1e30

_cached = {}


def _ts(i, n):
    return slice(i * n, (i + 1) * n)


def build_bass():
    nc = bacc.Bacc("TRN2", target_bir_lowering=False, debug=False, num_devices=1)

    inp = {}
    def di(name, shape, dt):
        inp[name] = nc.dram_tensor(name, list(shape), dt, kind="ExternalInput")
        return inp[name]

    # big tensors are flat (128, N) with per-partition-contiguous layout so
    # every load is 128 single-run descriptors
    di("xt", (128, NT * 16 * 512), BF16)  # x[b].T chunked (p, tt, cc, t)
    di("wqa", (128, 16 * QLR), BF16)      # wq_a.T chunked (p=c, cc, l)
    di("wkva", (128, 16 * (KVLR + DR)), BF16)
    di("wqbn", (128, 8 * HPG * DN), BF16)  # nope rows of wq_b (group), .T chunked by l
    di("wqbr", (128, 8 * HPG * DR), BF16)  # rope rows
    di("wkvbk", (128, 4 * HPG * DN), BF16)
    di("wkvbv", (128, 4 * HPG * DV), BF16)  # moving operand (p=lc, lc, hd)
    di("wout_l", (128, HPG * DIM), BF16)   # lhsT (p=hd within head, head, o)
    di("cosq", (64, T), BF16)              # [cos32; cos32]
    di("sinqs", (64, T), BF16)             # [-sin32; +sin32]
    di("perm64", (64, 64), BF16)           # rotate-half swap lhsT
    di("maskt", (128, 4, 512), F32)        # additive causal masks, variant v=qt%4
    di("dep", (128, 16), F32)             # chain-dependency token (timing harness)
    di("bqa_t", (128, 8), F32)
    di("gq_t", (128, 8), F32)
    di("bq_t", (128, 8), F32)
    di("bqbn_t", (128, HPG), F32)
    di("bqbr_t", (64, HPG), F32)
    di("bkva_t", (128, 5), F32)            # 576 rows chunked, last chunk rows 0:64
    di("bkvbk_t", (128, HPG), F32)
    di("bkvbv_row", (1, HPG * DV), F32)    # v bias as row (broadcast over partitions)

    outp = nc.dram_tensor("outp", [DIM, T], F32, kind="ExternalOutput")
    dep_out = nc.dram_tensor("dep_out", [128, 16], F32, kind="ExternalOutput")

    with tile.TileContext(nc) as tc, ExitStack() as es:
        cst = es.enter_context(tc.tile_pool(name="cst", bufs=1))
        dram = es.enter_context(tc.tile_pool(name="dram", bufs=1, space="DRAM"))
        pD = es.enter_context(tc.tile_pool(name="pD", bufs=1))    # qln, kvl, krope (P1->P2)

        # ---- small constants (live whole kernel) ----
        ones_bf = cst.tile([128, 1], BF16)
        nc.vector.memset(ones_bf[:], 1.0)
        eps_t = cst.tile([1, 1], F32)
        nc.vector.memset(eps_t[:], EPS)
        perm = cst.tile([64, 64], BF16)
        nc.sync.dma_start(out=perm[:], in_=inp["perm64"][:, :])
        dep_t = cst.tile([128, 16], F32)
        nc.sync.dma_start(out=dep_t[:], in_=inp["dep"][:, :])
        nc.sync.dma_start(out=dep_out[:, :], in_=dep_t[:])
        bias_t = {}
        for nm, shape in [("bqa_t", (128, 8)), ("gq_t", (128, 8)), ("bq_t", (128, 8)),
                          ("bqbn_t", (128, HPG)), ("bqbr_t", (64, HPG)),
                          ("bkva_t", (128, 5)), ("bkvbk_t", (128, HPG))]:
            bias_t[nm] = cst.tile(list(shape), F32, tag=nm, name=nm)
            nc.sync.dma_start(out=bias_t[nm][:], in_=inp[nm][:, :])

        # ---- persistent intermediates (P1 -> P2) ----
        qln = pD.tile([128, 8, T], BF16)      # q_lora (raw then layernormed in place)
        kvl = pD.tile([128, 4, T], BF16)      # kv_lora
        krope_raw = pD.tile([64, T], BF16)    # decoupled k rope input (pre-rotation)

        # P2 weights/tables: loaded on the scalar queue right after the P1
        # weights (see below) so P2 never waits on them
        pW = es.enter_context(tc.tile_pool(name="pW", bufs=1))
        wkvbv = pW.tile([128, 4, HPG * DV], BF16)
        vb_bc = pW.tile([128, HPG * DV], F32)
        wkvbk = pW.tile([128, 4, HPG * DN], BF16)
        cosq = pW.tile([64, T], BF16)
        sinqs = pW.tile([64, T], BF16)
        wqbn = pW.tile([128, 8, HPG * DN], BF16)
        wqbr = pW.tile([128, 8, HPG * DR], BF16)

        # ================= P1: LoRA projections (q + kv share x tiles) =========
        # LN mean/rstd rows are broadcast to 128 partitions with a PE
        # ones-matmul into PSUM (bf16 rows; no DRAM roundtrip), so the LN
        # apply for tile tt can start as soon as tile tt's stats are done.
        with tc.tile_pool(name="w1", bufs=1) as w1, \
             tc.tile_pool(name="xpa", bufs=2) as xpa, \
             tc.tile_pool(name="p1e", bufs=3) as p1e, \
             tc.tile_pool(name="bcps", bufs=1, space="PSUM") as bcps_pool, \
             tc.tile_pool(name="p1ps", bufs=4, space="PSUM") as p1ps, \
             tc.tile_pool(name="stps", bufs=1, space="PSUM") as stps:
            wqa = w1.tile([128, 16, QLR], BF16)
            # first two cc-chunks load separately so the very first matmul
            # can start after only 0.5 MB of weight transfer
            nc.scalar.dma_start(out=wqa[:, 0:2, :], in_=inp["wqa"][:, 0:2 * QLR])
            nc.scalar.dma_start(out=wqa[:, 2:4, :], in_=inp["wqa"][:, 2 * QLR:4 * QLR])
            for c4 in range(1, 4):
                nc.scalar.dma_start(out=wqa[:, _ts(c4, 4), :],
                                    in_=inp["wqa"][:, c4 * 4 * QLR:(c4 + 1) * 4 * QLR])
            wkva = w1.tile([128, 16, KVLR + DR], BF16)
            nc.scalar.dma_start(out=wkva[:], in_=inp["wkva"][:, :])
            # P2 weights follow on the same queue; they have all of P1 to land
            nc.scalar.dma_start(out=wkvbv[:, :, :], in_=inp["wkvbv"][:, :])
            nc.scalar.dma_start(out=vb_bc[:], in_=inp["bkvbv_row"][:, :].to_broadcast([128, HPG * DV]))
            nc.scalar.dma_start(out=wkvbk[:, :, :], in_=inp["wkvbk"][:, :])
            nc.scalar.dma_start(out=cosq[:], in_=inp["cosq"][:, :])
            nc.scalar.dma_start(out=sinqs[:], in_=inp["sinqs"][:, :])
            nc.scalar.dma_start(out=wqbn[:, :, :], in_=inp["wqbn"][:, :])
            nc.scalar.dma_start(out=wqbr[:, :, :], in_=inp["wqbr"][:, :])
            ones_row = cst.tile([1, 128], BF16, tag="ones_row", name="ones_row")
            nc.vector.memset(ones_row[:], 1.0)
            for tt in range(NT):
                ts = _ts(tt, 512)
                xtile = xpa.tile([128, 16, 512], BF16, tag="xt")
                if tt == 0:
                    nc.sync.dma_start(out=xtile[:, 0:2, :],
                                      in_=inp["xt"][:, 0:2 * 512])
                    nc.sync.dma_start(out=xtile[:, 2:4, :],
                                      in_=inp["xt"][:, 2 * 512:4 * 512])
                    c4s = range(1, 4)
                else:
                    c4s = range(4)
                for c4 in c4s:
                    nc.sync.dma_start(out=xtile[:, _ts(c4, 4), :],
                                      in_=inp["xt"][:, (tt * 16 + c4 * 4) * 512:(tt * 16 + c4 * 4 + 4) * 512])
                stats = stps.tile([1, 1024], F32)

                def q_post(lc, ps):
                    nc.scalar.activation(out=qln[:, lc, ts], in_=ps[:], func=AF.Identity,
                                         bias=bias_t["bqa_t"][:, lc:lc + 1])
                    sq = p1e.tile([128, 512], BF16, tag="sq")
                    nc.vector.tensor_mul(sq[:], qln[:, lc, ts], qln[:, lc, ts])
                    nc.tensor.matmul(stats[:, 0:512], ones_bf[:], qln[:, lc, ts],
                                     start=(lc == 0), stop=(lc == 7))
                    nc.tensor.matmul(stats[:, 512:1024], ones_bf[:], sq[:],
                                     start=(lc == 0), stop=(lc == 7))

                if tt == 0:
                    # first tile: cc-outer in 4-lc groups so compute tracks the
                    # progressive arrival of the wqa/x chunks instead of
                    # stalling on the full 4 MB weight load
                    for half in range(2):
                        pss = [(half * 4 + j,
                                p1ps.tile([128, 512], F32, tag="p1ps", name="ps"))
                               for j in range(4)]
                        for cc in range(16):
                            for lc, ps in pss:
                                nc.tensor.matmul(ps[:], wqa[:, cc, _ts(lc, 128)],
                                                 xtile[:, cc, :],
                                                 start=(cc == 0), stop=(cc == 15))
                        for lc, ps in pss:
                            q_post(lc, ps)
                else:
                    for lc in range(8):
                        ps = p1ps.tile([128, 512], F32, tag="p1ps")
                        for cc in range(16):
                            nc.tensor.matmul(ps[:], wqa[:, cc, _ts(lc, 128)], xtile[:, cc, :],
                                             start=(cc == 0), stop=(cc == 15))
                        q_post(lc, ps)
                r1 = p1e.tile([1, 512], F32, tag="r1")
                r2 = p1e.tile([1, 512], F32, tag="r2")
                mrow_bf = p1e.tile([1, 512], BF16, tag="mrow_bf")
                rrow_bf = p1e.tile([1, 512], BF16, tag="rrow_bf")
                mrow_f = p1e.tile([1, 512], F32, tag="mrow_f")
                nc.vector.tensor_scalar_mul(mrow_f[:], stats[0:1, 0:512], 1.0 / QLR)
                nc.vector.tensor_scalar_mul(r1[:], stats[0:1, 512:1024], 1.0 / QLR)
                nc.vector.tensor_mul(r2[:], mrow_f[:], mrow_f[:])
                nc.vector.tensor_sub(r1[:], r1[:], r2[:])          # var
                nc.scalar.activation(out=r2[:], in_=r1[:], func=AF.Sqrt, bias=eps_t[:])
                with nc.allow_low_precision(reason="LN row broadcast via PE"):
                    nc.vector.reciprocal(out=rrow_bf[:], in_=r2[:])
                    nc.vector.tensor_copy(out=mrow_bf[:], in_=mrow_f[:])
                bcps = bcps_pool.tile([128, 1024], F32, tag="bc", name="bc")
                for oc in range(5):
                    rows_n = 128 if oc < 4 else 64
                    ps = p1ps.tile([128, 512], F32, tag="p1ps")
                    for cc in range(16):
                        nc.tensor.matmul(ps[:rows_n, :],
                                         wkva[:, cc, oc * 128:oc * 128 + rows_n],
                                         xtile[:, cc, :], start=(cc == 0), stop=(cc == 15))
                    if oc < 4:
                        nc.scalar.activation(out=kvl[:, oc, ts], in_=ps[:], func=AF.Identity,
                                             bias=bias_t["bkva_t"][:, oc:oc + 1])
                    else:
                        nc.scalar.activation(out=krope_raw[:, ts], in_=ps[:64, :],
                                             func=AF.Identity,
                                             bias=bias_t["bkva_t"][0:64, 4:5])
                    if oc == 0:
                        # broadcast the LN rows here: the DVE row math had the
                        # first kv matmul group to finish under, and the LN
                        # apply gets the rest of the kv phase to run in
                        nc.tensor.matmul(bcps[:, 0:512], ones_row[:], mrow_bf[:],
                                         start=True, stop=True)
                        nc.tensor.matmul(bcps[:, 512:1024], ones_row[:], rrow_bf[:],
                                         start=True, stop=True)
                # LN apply for this tile (DVE work overlapping the kv matmuls)
                for lc in range(8):
                    t1 = p1e.tile([128, 512], BF16, tag="lnt")
                    nc.vector.tensor_sub(t1[:], qln[:, lc, ts], bcps[:, 0:512])
                    nc.vector.tensor_mul(t1[:], t1[:], bcps[:, 512:1024])
                    nc.scalar.activation(out=qln[:, lc, ts], in_=t1[:], func=AF.Identity,
                                         scale=bias_t["gq_t"][:, lc:lc + 1],
                                         bias=bias_t["bq_t"][:, lc:lc + 1])

        # ================= P2: up-projections + rope =================
        pG = es.enter_context(tc.tile_pool(name="pG", bufs=1))    # q/k/v heads (P2->P3)
        qnope = pG.tile([128, HPG, T], BF16)
        qrope = pG.tile([64, HPG, T], BF16)
        knope = pG.tile([128, HPG, T], BF16)
        vtm = pG.tile([128, NQT, HPG * DV], BF16)   # V token-major (k, kt, hd)
        kr = pG.tile([64, T], BF16)                 # rotated k rope

        with tc.tile_pool(name="p2e", bufs=4) as p2e, \
             tc.tile_pool(name="p2ps", bufs=3, space="PSUM") as p2ps, \
             tc.tile_pool(name="p2ps64", bufs=2, space="PSUM") as p2ps64:

            def rope_block(dst_ap, src_ap, ts):
                """dst = rotate_half(src) in feature-major layout, (64, 512) block."""
                sw = p2ps64.tile([64, 512], F32, tag="swap", name="sw")
                nc.tensor.matmul(sw[:], perm[:], src_ap, start=True, stop=True)
                ta = p2e.tile([64, 512], F32, tag="ropea", name="ta")
                nc.vector.tensor_mul(ta[:], src_ap, cosq[:, ts])
                tb = p2e.tile([64, 512], F32, tag="ropeb", name="tb")
                nc.vector.tensor_mul(tb[:], sw[:], sinqs[:, ts])
                nc.vector.tensor_add(dst_ap, ta[:], tb[:])

            # kv-side first: independent of the layernorm chain, keeps PE busy
            # while the LN rows/broadcast latency resolves.
            for kt in range(NQT):
                ps = p2ps.tile([128, 512], F32, tag="p2ps", name="ps")
                for lc in range(4):
                    nc.tensor.matmul(ps[:], kvl[:, lc, _ts(kt, 128)], wkvbv[:, lc, :],
                                     start=(lc == 0), stop=(lc == 3))
                nc.vector.tensor_add(vtm[:, kt, :], ps[:], vb_bc[:])
            for tt in range(NT):
                ts = _ts(tt, 512)
                rope_block(kr[:, ts], krope_raw[:, ts], ts)
            for h in range(HPG):
                for tt in range(NT):
                    ts = _ts(tt, 512)
                    ps = p2ps.tile([128, 512], F32, tag="p2ps", name="ps")
                    for lc in range(4):
                        nc.tensor.matmul(ps[:], wkvbk[:, lc, _ts(h, DN)], kvl[:, lc, ts],
                                         start=(lc == 0), stop=(lc == 3))
                    nc.scalar.activation(out=knope[:, h, ts], in_=ps[:], func=AF.Identity,
                                         bias=bias_t["bkvbk_t"][:, h:h + 1])
            for tt in range(NT):
                for h in range(HPG):
                    ts = _ts(tt, 512)
                    # q nope
                    ps = p2ps.tile([128, 512], F32, tag="p2ps", name="ps")
                    for lc in range(8):
                        nc.tensor.matmul(ps[:], wqbn[:, lc, _ts(h, DN)], qln[:, lc, ts],
                                         start=(lc == 0), stop=(lc == 7))
                    nc.scalar.activation(out=qnope[:, h, ts], in_=ps[:], func=AF.Identity,
                                         bias=bias_t["bqbn_t"][:, h:h + 1])
                    # q rope
                    ps64 = p2ps64.tile([64, 512], F32, tag="qr", name="ps64")
                    for lc in range(8):
                        nc.tensor.matmul(ps64[:], wqbr[:, lc, _ts(h, DR)], qln[:, lc, ts],
                                         start=(lc == 0), stop=(lc == 7))
                    qr_raw = p2e.tile([64, 512], BF16, tag="qr_raw", name="qr_raw")
                    nc.scalar.activation(out=qr_raw[:], in_=ps64[:], func=AF.Identity,
                                         bias=bias_t["bqbr_t"][:, h:h + 1])
                    rope_block(qrope[:, h, ts], qr_raw[:], ts)

        # ================= P3: causal attention =================
        pI = es.enter_context(tc.tile_pool(name="pI", bufs=1))
        yt = pI.tile([128, HPG, T], BF16)           # attention out, feature-major
        wout_l = pI.tile([128, HPG, DIM], BF16)     # P4 weights, prefetched during P3
        nc.scalar.dma_start(out=wout_l[:, :, :], in_=inp["wout_l"][:, :])
        idb = pI.tile([128, 128], BF16)
        make_identity(nc, idb[:])

        with tc.tile_pool(name="amask", bufs=1) as amask, \
             tc.tile_pool(name="ap_s", bufs=3) as ap_s, \
             tc.tile_pool(name="ap_l", bufs=4) as ap_l, \
             tc.tile_pool(name="sps", bufs=3, space="PSUM") as spsp, \
             tc.tile_pool(name="ptps", bufs=2, space="PSUM") as ptps, \
             tc.tile_pool(name="yps", bufs=2, space="PSUM") as ypsp, \
             tc.tile_pool(name="ytps", bufs=1, space="PSUM") as ytpsp:
            maskt = amask.tile([128, 4, 512], F32)
            nc.sync.dma_start(out=maskt[:], in_=inp["maskt"][:, :, :])

            for h in range(HPG):
                for qt in range(NQT):
                    nkt = qt // 4 + 1
                    qs = _ts(qt, 128)
                    yps = ypsp.tile([128, 128], F32, tag="yacc", name="yps")
                    lpart = ap_l.tile([128, 4], F32, tag="lpart", name="lpart")
                    for kt in range(nkt):
                        # diagonal tile only covers its valid key width
                        diag = kt == qt // 4
                        w = (qt % 4 + 1) * 128 if diag else 512
                        nsub = qt % 4 + 1 if diag else 4
                        ks = slice(kt * 512, kt * 512 + w)
                        sps = spsp.tile([128, 512], F32, tag="sps", name="sps")
                        nc.tensor.matmul(sps[:, :w], qnope[:, h, qs], knope[:, h, ks],
                                         start=True, stop=False)
                        nc.tensor.matmul(sps[:, :w], qrope[:, h, qs], kr[:, ks],
                                         start=False, stop=True)
                        if diag:
                            nc.vector.tensor_add(sps[:, :w], sps[:, :w],
                                                 maskt[:, qt % 4, :w])
                        pbf = ap_s.tile([128, 512], BF16, tag="pbf", name="pbf")
                        nc.scalar.activation(out=pbf[:, :w], in_=sps[:, :w], func=AF.Exp,
                                             scale=SCALE,
                                             accum_out=lpart[:, kt:kt + 1])
                        ptp = ptps.tile([128, 512], BF16, tag="ptp", name="ptp")
                        for i in range(nsub):
                            nc.tensor.transpose(ptp[:, _ts(i, 128)], pbf[:, _ts(i, 128)], idb[:])
                        pts = ap_s.tile([128, 512], BF16, tag="pts", name="pts")
                        nc.vector.tensor_copy(out=pts[:, :w], in_=ptp[:, :w])
                        for i in range(nsub):
                            nc.tensor.matmul(yps[:], pts[:, _ts(i, 128)],
                                             vtm[:, kt * 4 + i, _ts(h, DV)],
                                             start=(kt == 0 and i == 0),
                                             stop=(kt == nkt - 1 and i == nsub - 1))
                    lsum = ap_l.tile([128, 1], F32, tag="lsum", name="lsum")
                    nc.vector.tensor_reduce(lsum[:], lpart[:, 0:nkt],
                                            axis=mybir.AxisListType.X, op=ALU.add)
                    linv = ap_l.tile([128, 1], F32, tag="linv", name="linv")
                    nc.vector.reciprocal(out=linv[:], in_=lsum[:])
                    ytmb = ap_s.tile([128, 128], BF16, tag="ytmb", name="ytmb")
                    nc.vector.tensor_scalar_mul(ytmb[:], yps[:], linv[:])
                    ytp = ytpsp.tile([128, 128], BF16, tag="ytp", name="ytp")
                    nc.tensor.transpose(ytp[:], ytmb[:], idb[:])
                    nc.vector.tensor_copy(out=yt[:, h, qs], in_=ytp[:])

        # ================= P4: output projection (partial) =================
        with tc.tile_pool(name="p4e", bufs=4) as p4e, \
             tc.tile_pool(name="p4ps", bufs=4, space="PSUM") as p4ps:
            for oc in range(16):
                for tt in range(NT):
                    ts = _ts(tt, 512)
                    ps = p4ps.tile([128, 512], F32, tag="p4ps", name="ps")
                    for h in range(HPG):
                        nc.tensor.matmul(ps[:], wout_l[:, h, _ts(oc, 128)], yt[:, h, ts],
                                         start=(h == 0), stop=(h == HPG - 1))
                    ot = p4e.tile([128, 512], F32, tag="ot", name="ot")
                    nc.scalar.copy(out=ot[:], in_=ps[:])
                    nc.sync.dma_start(out=outp[_ts(oc, 128), ts], in_=ot[:])

    nc.compile()
    return nc


def _chunk(a, p=128):
    """(N, M) -> (p, N//p, M) with chunk index as middle dim."""
    n, m = a.shape
    return np.ascontiguousarray(a.reshape(n // p, p, m).swapaxes(0, 1))


def _prep_inputs(x, wq_a, bq_a, g_q, b_q, wq_b, bq_b, wkv_a, bkv_a, wkv_b, bkv_b,
                 wout, bout):
    bf = lambda a: np.ascontiguousarray(a).astype(BF)
    f32 = lambda a: np.ascontiguousarray(a).astype(np.float32)

    # rope tables (feature-major), one 64-row head block
    inv = 1.0 / (10000.0 ** (np.arange(0, DR, 2, dtype=np.float64) / DR))
    ang = np.arange(T, dtype=np.float64)[:, None] * inv[None, :]      # (T, 32)
    cos32 = np.cos(ang).T                                             # (32, T)
    sin32 = np.sin(ang).T
    cosq = bf(np.concatenate([cos32, cos32], axis=0))
    sinqs = bf(np.concatenate([-sin32, sin32], axis=0))
    perm = np.zeros((64, 64), dtype=np.float32)
    for m in range(64):
        perm[(m + 32) % 64, m] = 1.0   # swapped[m] = x[m+32 mod 64]
    perm = bf(perm)

    maskt = np.zeros((128, 4, 512), dtype=np.float32)
    for v in range(4):
        for p in range(128):
            maskt[p, v, v * 128 + p + 1:] = MASK_NEG

    wq_b3 = wq_b.reshape(H, DQK, QLR)
    wkv_b3 = wkv_b.reshape(H, DN + DV, KVLR)
    bq_b3 = bq_b.reshape(H, DQK)
    bkv_b3 = bkv_b.reshape(H, DN + DV)

    bkva_pad = np.zeros((640,), dtype=np.float32)
    bkva_pad[:KVLR + DR] = bkv_a

    shared = {
        "wqa": _chunk(bf(wq_a.T)).reshape(128, -1),
        "wkva": _chunk(bf(wkv_a.T)).reshape(128, -1),
        "cosq": cosq, "sinqs": sinqs, "perm64": perm, "maskt": maskt,
        "bqa_t": f32(bq_a.reshape(8, 128).T),
        "gq_t": f32(g_q.reshape(8, 128).T),
        "bq_t": f32(b_q.reshape(8, 128).T),
        "bkva_t": f32(bkva_pad.reshape(5, 128).T),
        "dep": np.zeros((128, 16), np.float32),
    }

    # batch-level and group-level arrays are shared across cores: compute once
    # xt layout (128, NT, 16, 512): per-token-tile loads are one DMA with
    # 16KB-contiguous runs per partition
    xt_by_batch = {
        b: np.ascontiguousarray(
            _chunk(bf(x[b].T)).reshape(128, 16, NT, 512).swapaxes(1, 2)).reshape(128, -1)
        for b in range(B)
    }
    group_arrs = {}
    for g in range(HPG):  # 4 head groups
        hs = list(range(g * HPG, (g + 1) * HPG))
        wqbr_g = np.concatenate([wq_b3[h, :DR, :] for h in hs], axis=0)      # (256, QLR)
        wqbn_g = np.concatenate([wq_b3[h, DR:, :] for h in hs], axis=0)      # (512, QLR)
        wkvbk_g = np.concatenate([wkv_b3[h, :DN, :] for h in hs], axis=0)    # (512, KVLR)
        wkvbv_g = np.concatenate([wkv_b3[h, DN:, :] for h in hs], axis=0)    # (512, KVLR)
        wout_g = wout[:, g * HPG * DV:(g + 1) * HPG * DV]                    # (DIM, 512)
        group_arrs[g] = {
            "wqbn": _chunk(bf(wqbn_g.T)).reshape(128, -1),
            "wqbr": _chunk(bf(wqbr_g.T)).reshape(128, -1),
            "wkvbk": _chunk(bf(wkvbk_g.T)).reshape(128, -1),
            "wkvbv": _chunk(bf(wkvbv_g.T)).reshape(128, -1),
            "wout_l": _chunk(bf(np.ascontiguousarray(wout_g.T))).reshape(128, -1),
            "bqbn_t": f32(np.stack([bq_b3[h, DR:] for h in hs], axis=1)),    # (128, 4)
            "bqbr_t": f32(np.stack([bq_b3[h, :DR] for h in hs], axis=1)),    # (64, 4)
            "bkvbk_t": f32(np.stack([bkv_b3[h, :DN] for h in hs], axis=1)),
            "bkvbv_row": f32(np.concatenate([bkv_b3[h, DN:] for h in hs])[None, :]),
        }
    in_maps = []
    for c in range(NCORES):
        b, g = divmod(c, HPG)
        m = dict(shared)
        m["xt"] = xt_by_batch[b]
        m.update(group_arrs[g])
        in_maps.append(m)
    return in_maps


def kernel(**inputs):
    inputs = {k: np.asarray(v) for k, v in inputs.items()}
    in_maps = _prep_inputs(**inputs)
    if "nc" not in _cached:
        _cached["nc"] = build_bass()
    res = run_bass_kernel_spmd(_cached["nc"], in_maps, core_ids=list(range(NCORES)))
    bout = inputs["bout"].astype(np.float64)
    out = np.zeros((B, T, DIM), dtype=np.float64)
    for c in range(NCORES):
        b = c // HPG
        out[b] += res.results[c]["outp"].astype(np.float64).T
    out += bout[None, None, :]
    return out.astype(np.float32)


if __name__ == "__main__":
    rng = np.random.default_rng(0)
    dummy = {
        "x": rng.standard_normal((B, T, DIM), dtype=np.float32),
        "wq_a": rng.standard_normal((QLR, DIM), dtype=np.float32) * 0.02,
        "bq_a": np.zeros(QLR, np.float32),
        "g_q": np.ones(QLR, np.float32),
        "b_q": np.zeros(QLR, np.float32),
        "wq_b": rng.standard_normal((H * DQK, QLR), dtype=np.float32) * 0.02,
        "bq_b": np.zeros(H * DQK, np.float32),
        "wkv_a": rng.standard_normal((KVLR + DR, DIM), dtype=np.float32) * 0.02,
        "bkv_a": np.zeros(KVLR + DR, np.float32),
        "wkv_b": rng.standard_normal((H * (DN + DV), KVLR), dtype=np.float32) * 0.02,
        "bkv_b": np.zeros(H * (DN + DV), np.float32),
        "wout": rng.standard_normal((DIM, DIM), dtype=np.float32) * 0.02,
        "bout": np.zeros(DIM, np.float32),
    }
    out = kernel(**dummy)
    print("out", out.shape, out.dtype, np.abs(out).max())

